# revision 9
# baseline (speedup 1.0000x reference)
"""Contrastive loss (SupCon-style) on 8 Trainium2 NeuronCores.

Reference (N=8192, D=1024, T=0.1):
    sim = emb @ emb.T / T;  e = exp(sim)
    all_sum_i = sum_j e_ij - e_ii
    pos_sum_i = sum_j e_ij * lab_j - e_ii * lab_i
    loss = mean_{i: lab_i=1} [ log(all_sum_i + eps) - log(pos_sum_i) ]
    (0.0 if n_ref < 2)

Symmetric decomposition: e is symmetric, so each ordered pair (i, j) is
computed once. Rows are split across 8 cores (1024 rows each); each core
gets a column-ROTATED window of emb^T so its rows sit at local columns
[0, 1024). For local row-block bi (128 rows), the core computes the strip
of 33 column blocks [bi, bi+33) -- a half-circle sliding window. Every
unordered pair lands in exactly one strip, except block-distance-32 pairs
which land in two (on different cores); those tiles are halved exactly by
adding ln(0.5) to the exp argument. The diagonal block gets -BIG at the
self positions pre-exp (exact self-exclusion) and is excluded from column
sums (its transpose pairs are already in the row sums).

Per strip chunk [128 x 1024]: 8 accumulating fp8 DoubleRow matmuls;
ScalarE exp with accum_out -> row-sum partials; VectorE
scalar_tensor_tensor(exp * lab_j) -> pos row-sum partials; a tiny
[ones | lab_i] 2-column stationary matmul over the exp strip -> column
all/pos sums in one pass. All partials DMA to DRAM; the host combines the
O(N) partials (unshard step) and finishes the loss in fp64.
"""

import numpy as np

import concourse.bass as bass
import concourse.tile as tile
import concourse.mybir as mybir
from concourse import bacc
from concourse.bass_utils import run_bass_kernel_spmd

N, D = 8192, 1024
NCORES = 8
ROWS = N // NCORES  # 1024 rows per core
P = 128             # partitions
IC = ROWS // P      # 8 row blocks per core
ND = D // P         # 8 contraction chunks
WIN = 33            # window blocks per row block (incl diag + dist-32)
WCOLS = (IC - 1 + WIN) * P      # 5120: union of all strips on this core
NCH = 5             # strip chunks: 4 x 1024 + 1 x 128 (the dist-32 tail)
CSTRIP = (WIN - 1) * P          # 4096 col-sum columns per bi (excl diag)
SCALE = 10.0        # 1 / TEMPERATURE
EPS = 1e-8
BIG = 1e9           # sim[diag] -= BIG before exp => exp -> 0
LN_HALF = float(np.log(0.5))

F32 = mybir.dt.float32
BF16 = mybir.dt.bfloat16
DT_MM = mybir.dt.float8e4

_build_cache = {}


def build(reps: int = 1, dt_mm=None):
    if dt_mm is None:
        dt_mm = DT_MM
    key = (reps, dt_mm)
    if key in _build_cache:
        return _build_cache[key]

    nc = bacc.Bacc("TRN2", target_bir_lowering=False, debug=False)
    embW_d = nc.dram_tensor("embW", [D, WCOLS], dt_mm, kind="ExternalInput")
    labw_d = nc.dram_tensor("labw", [WCOLS], BF16, kind="ExternalInput")
    colw_d = nc.dram_tensor("colw", [P, IC, 2], BF16, kind="ExternalInput")
    alls_d = nc.dram_tensor("alls", [P, IC * NCH], F32, kind="ExternalOutput")
    poss_d = nc.dram_tensor("poss", [P, IC * NCH], F32, kind="ExternalOutput")
    colp_d = nc.dram_tensor("colp", [2, IC * CSTRIP], F32, kind="ExternalOutput")

    # [D, WCOLS] viewed as [p, dc, n] with d = dc*128 + p
    embW = embW_d.ap().rearrange("(dc p) n -> p dc n", p=P)
    labw_bcast = bass.AP(tensor=labw_d, offset=0, ap=[[0, P], [1, WCOLS]])

    use_dr = dt_mm in mybir.MATMUL_PERF_MODE_DTYPES

    with tile.TileContext(nc) as tc:
        with (
            tc.tile_pool(name="consts", bufs=1) as consts,
            tc.tile_pool(name="expp", bufs=4) as expp,
            tc.tile_pool(name="scrp", bufs=2) as scrp,
            tc.tile_pool(name="cstg", bufs=6) as cstg,
            tc.tile_pool(name="stats", bufs=1) as stats,
            tc.tile_pool(name="psum", bufs=3, space=bass.MemorySpace.PSUM) as psum,
            tc.tile_pool(name="cpsum", bufs=2, space=bass.MemorySpace.PSUM) as cpsum,
        ):
            # resident window of rotated emb^T columns; chunked DMAs so the
            # bi=0 strip can start before the tail of the window arrives
            B = consts.tile([P, ND, WCOLS], dt_mm)
            for c0 in range(0, WCOLS, 1024):
                nc.sync.dma_start(out=B[:, :, c0 : c0 + 1024],
                                  in_=embW[:, :, c0 : c0 + 1024])
            # labels broadcast across partitions (bf16: 0/1 exact)
            labb = consts.tile([P, WCOLS], BF16)
            nc.gpsimd.dma_start(out=labb, in_=labw_bcast)
            # [ones | own-row labels] col-sum stationary operands
            colws = consts.tile([P, IC, 2], BF16)
            nc.gpsimd.dma_start(out=colws, in_=colw_d.ap())
            # BIG * identity (subtracted on the diagonal block pre-exp)
            bigI = consts.tile([P, P], F32)
            nc.gpsimd.memset(bigI, 0.0)
            nc.gpsimd.affine_select(
                out=bigI,
                in_=bigI,
                compare_op=mybir.AluOpType.not_equal,
                fill=BIG,
                base=0,
                pattern=[[-1, P]],
                channel_multiplier=1,
            )
            ln05 = consts.tile([P, 1], F32)
            nc.vector.memset(ln05, LN_HALF)
            zerob = consts.tile([P, 1], F32)
            nc.vector.memset(zerob, 0.0)

            for rep in range(reps):
                alls = stats.tile([P, IC * NCH], F32, tag="alls")
                poss = stats.tile([P, IC * NCH], F32, tag="poss")

                for bi in range(IC):
                    stat = B[:, :, bi * P : (bi + 1) * P]
                    for ch in range(NCH):
                        w = 1024 if ch < 4 else P
                        c0 = bi * P + ch * 1024
                        ps = psum.tile([P, 1024], F32, tag="ps")
                        for s in range(max(1, w // 512)):
                            sw = min(512, w - s * 512)
                            if use_dr:
                                for dp in range(ND // 2):
                                    nc.tensor.matmul(
                                        ps[:, s * 512 : s * 512 + sw],
                                        stat[:, 2 * dp : 2 * dp + 2, :],
                                        B[:, 2 * dp : 2 * dp + 2,
                                          c0 + s * 512 : c0 + s * 512 + sw],
                                        start=(dp == 0),
                                        stop=(dp == ND // 2 - 1),
                                        perf_mode=mybir.MatmulPerfMode.DoubleRow,
                                    )
                            else:
                                for dc in range(ND):
                                    nc.tensor.matmul(
                                        ps[:, s * 512 : s * 512 + sw],
                                        stat[:, dc, :],
                                        B[:, dc, c0 + s * 512 : c0 + s * 512 + sw],
                                        start=(dc == 0),
                                        stop=(dc == ND - 1),
                                    )
                        if ch == 0:
                            # diagonal block at strip cols [0, 128)
                            nc.vector.tensor_sub(ps[:, 0:P], ps[:, 0:P], bigI)
                        ext = expp.tile([P, 1024], BF16, tag="ext")
                        idx = bi * NCH + ch
                        nc.scalar.activation(
                            out=ext[:, 0:w],
                            in_=ps[:, 0:w],
                            func=mybir.ActivationFunctionType.Exp,
                            scale=SCALE,
                            bias=ln05 if ch == NCH - 1 else zerob,
                            accum_out=alls[:, idx : idx + 1],
                        )
                        junk = scrp.tile([P, 1024], BF16, tag="junk")
                        nc.vector.scalar_tensor_tensor(
                            out=junk[:, 0:w],
                            in0=ext[:, 0:w],
                            scalar=1.0,
                            in1=labb[:, c0 : c0 + w],
                            op0=mybir.AluOpType.mult,
                            op1=mybir.AluOpType.mult,
                            accum_out=poss[:, idx : idx + 1],
                        )
                        # column sums over the strip, excluding the diagonal
                        # block: strip-excl col = local col - (bi*P + 128)
                        pieces = []
                        if ch == 0:
                            pieces = [(P, 512), (P + 512, 384)]
                        elif ch < 4:
                            pieces = [(0, 512), (512, 512)]
                        else:
                            pieces = [(0, P)]
                        for pi, (off, pw) in enumerate(pieces):
                            cs = cpsum.tile([2, 512], F32, tag="cs")
                            nc.tensor.matmul(
                                cs[:, 0:pw],
                                colws[:, bi, :],
                                ext[:, off : off + pw],
                                start=True,
                                stop=True,
                            )
                            # PSUM -> SBUF (DMA can't read PSUM); alternate
                            # ACT/DVE so neither becomes the bottleneck
                            st = cstg.tile([2, 512], F32, tag="st")
                            if (ch + pi) % 2 == 0:
                                nc.scalar.copy(out=st[:, 0:pw], in_=cs[:, 0:pw])
                            else:
                                nc.vector.tensor_copy(out=st[:, 0:pw],
                                                      in_=cs[:, 0:pw])
                            dst = bi * CSTRIP + (c0 + off) - (bi * P + P)
                            nc.sync.dma_start(
                                out=colp_d.ap()[:, dst : dst + pw],
                                in_=st[:, 0:pw],
                            )

                nc.sync.dma_start(out=alls_d.ap(), in_=alls)
                nc.sync.dma_start(out=poss_d.ap(), in_=poss)

    nc.compile()
    _build_cache[key] = nc
    return nc


def make_in_maps(embeddings: np.ndarray, labels: np.ndarray, dt_mm=None):
    if dt_mm is None:
        dt_mm = DT_MM
    emb = np.asarray(embeddings, dtype=np.float32)
    lab_f = np.asarray(labels).astype(np.float32)
    embT = np.ascontiguousarray(emb.T)  # [D, N]
    np_dt = mybir.dt.np(dt_mm)
    np_bf = mybir.dt.np(BF16)
    in_maps = []
    for c in range(NCORES):
        embW = np.roll(embT, -c * ROWS, axis=1)[:, :WCOLS]
        labw = np.roll(lab_f, -c * ROWS)[:WCOLS]
        own = lab_f[c * ROWS : (c + 1) * ROWS].reshape(IC, P).T  # [P, IC]
        colw = np.empty((P, IC, 2), np.float32)
        colw[:, :, 0] = 1.0
        colw[:, :, 1] = own
        in_maps.append(
            {
                "embW": np.ascontiguousarray(embW).astype(np_dt),
                "labw": labw.astype(np_bf),
                "colw": colw.astype(np_bf),
            }
        )
    return in_maps


def finish(outs: list, labels: np.ndarray) -> np.ndarray:
    """Combine per-core partial sums (the unshard step) and finish in fp64."""
    lab = np.asarray(labels).astype(np.float64)
    n_ref = lab.sum()
    if n_ref < 2:
        return np.float32(0.0)
    all_sum = np.zeros(N)
    pos_sum = np.zeros(N)
    k = np.arange(CSTRIP)
    for c in range(NCORES):
        alls = np.asarray(outs[c]["alls"], np.float64).reshape(P, IC, NCH)
        poss = np.asarray(outs[c]["poss"], np.float64).reshape(P, IC, NCH)
        colp = np.asarray(outs[c]["colp"], np.float64).reshape(2, IC, CSTRIP)
        # row partials: partition p of block bi is global row c*1024 + bi*128 + p
        rows = (c * ROWS + np.arange(ROWS)).reshape(IC, P).T  # [P, IC]
        np.add.at(all_sum, rows, alls.sum(axis=2))
        np.add.at(pos_sum, rows, poss.sum(axis=2))
        for bi in range(IC):
            gj = (c * ROWS + bi * P + P + k) % N
            np.add.at(all_sum, gj, colp[0, bi])
            np.add.at(pos_sum, gj, colp[1, bi])
    loss_per_row = np.log(all_sum + EPS) - np.log(pos_sum)
    loss = np.where(lab > 0, loss_per_row, 0.0).sum() / max(n_ref, 1.0)
    return np.asarray(loss, dtype=np.float32)


def kernel(embeddings: np.ndarray, labels: np.ndarray) -> np.ndarray:
    lab_f = np.asarray(labels).astype(np.float32)
    if lab_f.sum() < 2:
        return np.float32(0.0)
    nc = build(reps=1)
    in_maps = make_in_maps(embeddings, labels)
    res = run_bass_kernel_spmd(nc, in_maps, core_ids=list(range(NCORES)))
    return finish(res.results, labels)


# revision 14
# speedup vs baseline: 4.3902x; 4.3902x over previous
"""Contrastive loss (SupCon-style) on 8 Trainium2 NeuronCores.

Reference (N=8192, D=1024, T=0.1):
    sim = emb @ emb.T / T;  e = exp(sim)
    all_sum_i = sum_j e_ij - e_ii
    pos_sum_i = sum_j e_ij * lab_j - e_ii * lab_i
    loss = mean_{i: lab_i=1} [ log(all_sum_i + eps) - log(pos_sum_i) ]
    (0.0 if n_ref < 2)

Symmetric decomposition: e is symmetric, so each ordered pair (i, j) is
computed once. Rows are split across 8 cores (1024 rows each); each core
gets a column-ROTATED window of emb^T so its rows sit at local columns
[0, 1024). For local row-block bi (128 rows), the core computes the strip
of 33 column blocks [bi, bi+33) -- a half-circle sliding window. Every
unordered pair lands in exactly one strip, except block-distance-32 pairs
which land in two (on different cores); those tiles are halved exactly by
adding ln(0.5) to the exp argument. The diagonal block gets -BIG at the
self positions pre-exp (exact self-exclusion) and is excluded from column
sums (its transpose pairs are already in the row sums).

Row sums fall out of the ScalarE exp accum_out; pos row sums from a
VectorE scalar_tensor_tensor against broadcast labels. Column sums use a
transposed matmul per 128-col block: stationary = exp block, moving =
[ones | own-labels] (2 cols), so the output is [128, 2] -- partition
distributed -- and a whole strip's column sums land in one [128, 64] PSUM
bank, drained by a single cheap copy. Column-sum groups are emitted one
bi behind the main matmuls so the PE never waits on exp. All partials DMA
to DRAM; the host combines them (unshard) and finishes the loss in fp64.
"""

import numpy as np

import concourse.bass as bass
import concourse.tile as tile
import concourse.mybir as mybir
from concourse import bacc
from concourse.bass_utils import run_bass_kernel_spmd

N, D = 8192, 1024
NCORES = 8
ROWS = N // NCORES  # 1024 rows per core
P = 128             # partitions
IC = ROWS // P      # 8 row blocks per core
ND = D // P         # 8 contraction chunks
WIN = 33            # window blocks per row block (incl diag + dist-32)
WCOLS = (IC - 1 + WIN) * P      # 5120: union of all strips on this core
SCOLS = WIN * P                 # 4224 strip columns
NSLOT = 5           # row-sum accum slots per bi (3x1024 + 1024 + 128)
CW = WIN - 1        # 32 col-sum blocks per bi (excl diag)
SCALE = 10.0        # 1 / TEMPERATURE
EPS = 1e-8
BIG = 1e9           # sim[diag] -= BIG before exp => exp -> 0
LN_HALF = float(np.log(0.5))

F32 = mybir.dt.float32
BF16 = mybir.dt.bfloat16
DT_MM = mybir.dt.float8e4

# main-chunk column layout within a strip: [start, width]
CHUNKS = [(0, 1024), (1024, 1024), (2048, 1024), (3072, 1152)]

_build_cache = {}


def build(reps: int = 1, dt_mm=None, level: int = 4):
    """level: 0=mm+exp, 1=+stt, 2=+colsum mm, 3=+copies, 4=full (+drain DMA)."""
    if dt_mm is None:
        dt_mm = DT_MM
    key = (reps, dt_mm, level)
    if key in _build_cache:
        return _build_cache[key]

    nc = bacc.Bacc("TRN2", target_bir_lowering=False, debug=False)
    embW_d = nc.dram_tensor("embW", [D, WCOLS], dt_mm, kind="ExternalInput")
    labw_d = nc.dram_tensor("labw", [WCOLS], BF16, kind="ExternalInput")
    colw_d = nc.dram_tensor("colw", [P, IC, 2], BF16, kind="ExternalInput")
    alls_d = nc.dram_tensor("alls", [P, IC * NSLOT], F32, kind="ExternalOutput")
    poss_d = nc.dram_tensor("poss", [P, IC * NSLOT], F32, kind="ExternalOutput")
    colp_d = nc.dram_tensor("colp", [P, IC * CW * 2], F32, kind="ExternalOutput")

    # [D, WCOLS] viewed as [p, dc, n] with d = dc*128 + p
    embW = embW_d.ap().rearrange("(dc p) n -> p dc n", p=P)
    labw_bcast = bass.AP(tensor=labw_d, offset=0, ap=[[0, P], [1, WCOLS]])

    use_dr = dt_mm in mybir.MATMUL_PERF_MODE_DTYPES

    with tile.TileContext(nc) as tc:
        with (
            tc.tile_pool(name="consts", bufs=1) as consts,
            tc.tile_pool(name="expp", bufs=3) as expp,
            tc.tile_pool(name="scrp", bufs=2) as scrp,
            tc.tile_pool(name="cstg", bufs=2) as cstg,
            tc.tile_pool(name="stats", bufs=1) as stats,
            tc.tile_pool(name="psum", bufs=2, space=bass.MemorySpace.PSUM) as psum,
            tc.tile_pool(name="cpsum", bufs=2, space=bass.MemorySpace.PSUM) as cpsum,
        ):
            # resident window of rotated emb^T columns; chunked DMAs so the
            # bi=0 strip can start before the tail of the window arrives
            B = consts.tile([P, ND, WCOLS], dt_mm)
            for c0 in range(0, WCOLS, 1024):
                nc.sync.dma_start(out=B[:, :, c0 : c0 + 1024],
                                  in_=embW[:, :, c0 : c0 + 1024])
            # labels broadcast across partitions (bf16: 0/1 exact)
            labb = consts.tile([P, WCOLS], BF16)
            nc.gpsimd.dma_start(out=labb, in_=labw_bcast)
            # [ones | own-row labels] col-sum moving operands
            colws = consts.tile([P, IC, 2], BF16)
            nc.gpsimd.dma_start(out=colws, in_=colw_d.ap())
            # BIG * identity (subtracted on the diagonal block pre-exp)
            bigI = consts.tile([P, P], F32)
            nc.gpsimd.memset(bigI, 0.0)
            nc.gpsimd.affine_select(
                out=bigI,
                in_=bigI,
                compare_op=mybir.AluOpType.not_equal,
                fill=BIG,
                base=0,
                pattern=[[-1, P]],
                channel_multiplier=1,
            )
            ln05 = consts.tile([P, 1], F32)
            nc.vector.memset(ln05, LN_HALF)
            zerob = consts.tile([P, 1], F32)
            nc.vector.memset(zerob, 0.0)

            def emit_colsums(bi, ext):
                """Column sums of strip bi: one transposed matmul per block
                (stationary = exp block, moving = [ones|lab]) accumulating
                into a [P, 64] PSUM tile; one copy + one DMA to drain."""
                if level < 2:
                    return
                cps = cpsum.tile([P, CW * 2], F32, tag="cps")
                for wb in range(1, WIN):
                    nc.tensor.matmul(
                        cps[:, 2 * (wb - 1) : 2 * wb],
                        ext[:, wb * P : (wb + 1) * P],
                        colws[:, bi, :],
                        start=True,
                        stop=True,
                    )
                if level < 3:
                    return
                st = cstg.tile([P, CW * 2], F32, tag="st")
                if bi % 2 == 0:
                    nc.scalar.copy(out=st, in_=cps)
                else:
                    nc.vector.tensor_copy(out=st, in_=cps)
                if level < 4:
                    return
                dst = bi * CW * 2
                nc.sync.dma_start(
                    out=colp_d.ap()[:, dst : dst + CW * 2], in_=st
                )

            for rep in range(reps):
                alls = stats.tile([P, IC * NSLOT], F32, tag="alls")
                poss = stats.tile([P, IC * NSLOT], F32, tag="poss")

                pending = None  # (bi, ext) whose colsums are not yet emitted
                for bi in range(IC):
                    stat = B[:, :, bi * P : (bi + 1) * P]
                    ext = expp.tile([P, SCOLS], BF16, tag="ext")
                    for ch, (cc0, w) in enumerate(CHUNKS):
                        c0 = bi * P + cc0
                        ps = psum.tile([P, 1152], F32, tag="ps")
                        for s0 in range(0, w, 512):
                            sw = min(512, w - s0)
                            if use_dr:
                                for dp in range(ND // 2):
                                    nc.tensor.matmul(
                                        ps[:, s0 : s0 + sw],
                                        stat[:, 2 * dp : 2 * dp + 2, :],
                                        B[:, 2 * dp : 2 * dp + 2,
                                          c0 + s0 : c0 + s0 + sw],
                                        start=(dp == 0),
                                        stop=(dp == ND // 2 - 1),
                                        perf_mode=mybir.MatmulPerfMode.DoubleRow,
                                    )
                            else:
                                for dc in range(ND):
                                    nc.tensor.matmul(
                                        ps[:, s0 : s0 + sw],
                                        stat[:, dc, :],
                                        B[:, dc, c0 + s0 : c0 + s0 + sw],
                                        start=(dc == 0),
                                        stop=(dc == ND - 1),
                                    )
                        if ch == 0:
                            # diagonal block at strip cols [0, 128)
                            nc.vector.tensor_sub(ps[:, 0:P], ps[:, 0:P], bigI)
                        # exp passes; the last 128 cols (distance-32 block)
                        # get the ln(0.5) bias
                        if ch < 3:
                            parts = [(0, w, zerob, ch)]
                        else:
                            parts = [(0, 1024, zerob, 3), (1024, 128, ln05, 4)]
                        for po, pw, bias, slot in parts:
                            idx = bi * NSLOT + slot
                            nc.scalar.activation(
                                out=ext[:, cc0 + po : cc0 + po + pw],
                                in_=ps[:, po : po + pw],
                                func=mybir.ActivationFunctionType.Exp,
                                scale=SCALE,
                                bias=bias,
                                accum_out=alls[:, idx : idx + 1],
                            )
                            if level >= 1:
                                junk = scrp.tile([P, 1152], BF16, tag="junk")
                                nc.vector.scalar_tensor_tensor(
                                    out=junk[:, 0:pw],
                                    in0=ext[:, cc0 + po : cc0 + po + pw],
                                    scalar=1.0,
                                    in1=labb[:, c0 + po : c0 + po + pw],
                                    op0=mybir.AluOpType.mult,
                                    op1=mybir.AluOpType.mult,
                                    accum_out=poss[:, idx : idx + 1],
                                )
                    # software pipeline: colsums lag the main matmuls by one
                    # bi so the PE never waits on this strip's exp
                    if pending is not None:
                        emit_colsums(*pending)
                    pending = (bi, ext)
                if pending is not None:
                    emit_colsums(*pending)

                nc.sync.dma_start(out=alls_d.ap(), in_=alls)
                if level >= 1:
                    nc.sync.dma_start(out=poss_d.ap(), in_=poss)

    nc.compile()
    _build_cache[key] = nc
    return nc


def make_in_maps(embeddings: np.ndarray, labels: np.ndarray, dt_mm=None):
    if dt_mm is None:
        dt_mm = DT_MM
    emb = np.asarray(embeddings, dtype=np.float32)
    lab_f = np.asarray(labels).astype(np.float32)
    embT = np.ascontiguousarray(emb.T)  # [D, N]
    np_dt = mybir.dt.np(dt_mm)
    np_bf = mybir.dt.np(BF16)
    in_maps = []
    for c in range(NCORES):
        embW = np.roll(embT, -c * ROWS, axis=1)[:, :WCOLS]
        labw = np.roll(lab_f, -c * ROWS)[:WCOLS]
        own = lab_f[c * ROWS : (c + 1) * ROWS].reshape(IC, P).T  # [P, IC]
        colw = np.empty((P, IC, 2), np.float32)
        colw[:, :, 0] = 1.0
        colw[:, :, 1] = own
        in_maps.append(
            {
                "embW": np.ascontiguousarray(embW).astype(np_dt),
                "labw": labw.astype(np_bf),
                "colw": colw.astype(np_bf),
            }
        )
    return in_maps


def finish(outs: list, labels: np.ndarray) -> np.ndarray:
    """Combine per-core partial sums (the unshard step) and finish in fp64."""
    lab = np.asarray(labels).astype(np.float64)
    n_ref = lab.sum()
    if n_ref < 2:
        return np.float32(0.0)
    all_sum = np.zeros(N)
    pos_sum = np.zeros(N)
    for c in range(NCORES):
        alls = np.asarray(outs[c]["alls"], np.float64).reshape(P, IC, NSLOT)
        poss = np.asarray(outs[c]["poss"], np.float64).reshape(P, IC, NSLOT)
        colp = np.asarray(outs[c]["colp"], np.float64).reshape(P, IC, CW, 2)
        # row partials: partition p of block bi is global row c*1024 + bi*128 + p
        rows = (c * ROWS + np.arange(ROWS)).reshape(IC, P).T  # [P, IC]
        np.add.at(all_sum, rows, alls.sum(axis=2))
        np.add.at(pos_sum, rows, poss.sum(axis=2))
        # col partials: colp[p, bi, wb, t] is col (c*1024 + (bi+1+wb)*128 + p) % N
        p = np.arange(P)[:, None, None]
        bi = np.arange(IC)[None, :, None]
        wb = np.arange(CW)[None, None, :]
        gj = (c * ROWS + (bi + 1 + wb) * P + p) % N
        np.add.at(all_sum, gj, colp[:, :, :, 0])
        np.add.at(pos_sum, gj, colp[:, :, :, 1])
    loss_per_row = np.log(all_sum + EPS) - np.log(pos_sum)
    loss = np.where(lab > 0, loss_per_row, 0.0).sum() / max(n_ref, 1.0)
    return np.asarray(loss, dtype=np.float32)


def kernel(embeddings: np.ndarray, labels: np.ndarray) -> np.ndarray:
    lab_f = np.asarray(labels).astype(np.float32)
    if lab_f.sum() < 2:
        return np.float32(0.0)
    nc = build(reps=1)
    in_maps = make_in_maps(embeddings, labels)
    res = run_bass_kernel_spmd(nc, in_maps, core_ids=list(range(NCORES)))
    return finish(res.results, labels)
